# revision 3
# baseline (speedup 1.0000x reference)
"""Trainium2 Bass kernel for nn_SamplingBlock (gnn_message_passing).

Strategy
--------
8 cores = (batch b in 0..3) x (vertex half h in 0..1); each core owns 4096
vertices of one batch, fully data-parallel (no collectives).

Host-side weight folding (weights-only algebra, no data computation):
    M_k   = W_sum[:,:,k] @ W_diff          (k = 0..8; [256, 259])
    M_0  += W_center
    bias  = sum_k W_sum[:,:,k] @ b_diff + b_sum + b_center       ([256])
    out[n] = M_0 @ [xp_n; v_n; 1*] + sum_{k>=1} M_k @ [xn_{n,k}; nb_{n,k}]
(the trailing 1-row carries the bias; this removes the separate
sum_neighbourhood conv entirely - it fuses into one 260-row contraction).

The volume is re-laid out as an fp16 CELL table on the host: cell (z,y,x)
stores its 8 trilinear corners contiguously (8*256 fp16 = 4 KB), with edge
clamping baked in by the host. One dma_gather element therefore covers a
whole sample (1 descriptor pair/sample instead of 4, and half the bytes of
the old f32 row-pair layout).

Device pipeline per core (Tile framework), software-pipelined per
512-vertex chunk as centerA(vc+1) ahead of neighborsB(vc):
  centerA: one 2 MB center gather -> fp16 trilinear blend on DVE (fused
    custom op out=a*s0+b*s1, per-partition point scalars) -> PE transpose
    to K-major fp16 feature tiles (kept in SBUF for phase B) -> shift
    matmul -> neighbour coords -> index math -> ONE batched idx
    relayout (DRAM round-trip + rep16 matmul) for all 8 neighbours
  neighborsB: 8 independent 2 MB neighbour gathers stream back-to-back;
    per group: blend -> transpose -> 3 matmuls accumulating into the
    chunk's PSUM tile (27 matmuls total incl. the stored center k=0)
"""

import os
import sys

import numpy as np

for _p in ("/opt/trn_rl_repo", "/root/.axon_site/_ro/trn_rl_repo"):
    if os.path.isdir(_p) and _p not in sys.path:
        sys.path.insert(0, _p)
        break

import concourse.bacc as bacc
import concourse.bass as bass
import concourse.mybir as mybir
import concourse.tile as tile
from concourse.bass_utils import run_bass_kernel_spmd
from concourse.masks import make_identity

# ---------------------------------------------------------------- constants
B, N, C, NN = 4, 8192, 256, 8
GRID = 32
CELLS = GRID * GRID * GRID         # 32768 cells; idx fits int16 exactly
ESC = 8 * C                        # gather element: 8 corners x 256 ch fp16
NVC = N // 2                       # vertices per core = 4096
VCHUNK = 512                       # vertices per chunk
GPC = VCHUNK // 128                # groups (128-pt tiles) per chunk = 4
F32 = mybir.dt.float32
F16 = mybir.dt.float16
I16 = mybir.dt.int16
ALU = mybir.AluOpType
MM_DT = F16        # matmul operand dtype (full-rate on PE; fp32 would be 4x)

USE_CUSTOM_DVE = True

# ------------------------------------------------------- custom DVE op SCALE2
_SCALE2 = None


def _register_scale2():
    """out = in0*s0 + in1*s1 (per-partition scalars). Registered once."""
    global _SCALE2
    if _SCALE2 is not None or not USE_CUSTOM_DVE:
        return
    import concourse.dve_ops as dve_ops
    from concourse.dve_spec import C0, C1, Spec, Src0, Src1, lower
    from concourse.dve_uop import DveOpSpec

    for op in dve_ops.OPS:
        if op.name == "SCALE2_GS":
            _SCALE2 = op
            return
    spec = Spec(
        body=Src0 * C0 + Src1 * C1,
        reference=lambda in0, in1, s0, s1, imm2: in0 * s0 + in1 * s1,
    )
    shas = {}
    for ver in ("v3", "v4"):
        tmp = DveOpSpec(name="SCALE2_GS", opcode=0, uops=lower(spec, ver=ver),
                        rd1_en=True)
        shas[ver] = tmp.sha(ver)
    op = dve_ops.DveOp("SCALE2_GS", spec, subdim=False, uops_sha=shas)
    dve_ops.OPS.append(op)
    dve_ops._SUB_OPCODE_FOR_NAME[op.name] = len(dve_ops.OPS) - 1
    dve_ops.CUSTOM_DVE_SPECS[op.name] = spec
    _SCALE2 = op


# ------------------------------------------------------------- device program
def _emit_index_math(nc, sb, coords, npts_free, out_r16, out_w8):
    """coords: [128, npts_free, 3] f32 AP (normalized [-1,1] space, unclipped).
    Writes out_r16 [128, npts_free] int16 cell indices and
    out_w8 [128, npts_free, 8] f32 corner weights (order: c = dz*4+dy*2+dx)."""
    S = npts_free
    g = sb.tile([128, S, 3], F32, tag="ixg")
    # g = clip((c+1)*15.5, 0, 31)
    nc.vector.tensor_scalar(g[:], coords, 15.5, 15.5, op0=ALU.mult, op1=ALU.add)
    nc.vector.tensor_scalar(g[:], g[:], float(GRID - 1), 0.0, op0=ALU.min,
                            op1=ALU.max)
    # floor(g) robust to HW f32->int rounding mode: q = int(g); q -= (g < q)
    qi = sb.tile([128, S, 3], mybir.dt.int32, tag="ixq")
    nc.vector.tensor_copy(qi[:], g[:])
    i0 = sb.tile([128, S, 3], F32, tag="ixi")
    nc.vector.tensor_copy(i0[:], qi[:])
    frc = sb.tile([128, S, 3], F32, tag="ixf")
    nc.vector.tensor_tensor(frc[:], g[:], i0[:], op=ALU.subtract)  # g - q
    msk = sb.tile([128, S, 3], F32, tag="ixm")
    nc.vector.tensor_scalar(msk[:], frc[:], 0.0, None, op0=ALU.is_lt)
    nc.vector.tensor_tensor(i0[:], i0[:], msk[:], op=ALU.subtract)
    nc.vector.tensor_tensor(frc[:], g[:], i0[:], op=ALU.subtract)
    # cell = z*1024 + y*32 + x   (exact in f32; max 32767)
    r = sb.tile([128, S], F32, tag="ixr")
    nc.vector.tensor_scalar(r[:], i0[:, :, 2:3].squeeze(2), 1024.0, None,
                            op0=ALU.mult)
    t = sb.tile([128, S], F32, tag="ixt")
    nc.vector.tensor_scalar(t[:], i0[:, :, 1:2].squeeze(2), 32.0, None,
                            op0=ALU.mult)
    nc.vector.tensor_tensor(r[:], r[:], t[:], op=ALU.add)
    nc.vector.tensor_tensor(r[:], r[:], i0[:, :, 0:1].squeeze(2), op=ALU.add)
    nc.vector.tensor_copy(out_r16, r[:])
    # weights: a=fx, b=fy, c=fz
    inv = sb.tile([128, S, 3], F32, tag="ixv")   # 1-f
    nc.vector.tensor_scalar(inv[:], frc[:], -1.0, 1.0, op0=ALU.mult, op1=ALU.add)
    wzy = sb.tile([128, S, 4], F32, tag="ixw")
    # zy order: 00:(1-fy)(1-fz) 01:fy(1-fz) 10:(1-fy)fz 11:fy*fz
    yz = [(inv, inv), (frc, inv), (inv, frc), (frc, frc)]
    for k, (ysrc, zsrc) in enumerate(yz):
        nc.vector.tensor_tensor(
            wzy[:, :, k : k + 1].squeeze(2),
            ysrc[:, :, 1:2].squeeze(2),
            zsrc[:, :, 2:3].squeeze(2),
            op=ALU.mult,
        )
    for k in range(4):
        nc.vector.tensor_tensor(
            out_w8[:, :, 2 * k : 2 * k + 1].squeeze(2),
            wzy[:, :, k : k + 1].squeeze(2),
            inv[:, :, 0:1].squeeze(2), op=ALU.mult)
        nc.vector.tensor_tensor(
            out_w8[:, :, 2 * k + 1 : 2 * k + 2].squeeze(2),
            wzy[:, :, k : k + 1].squeeze(2),
            frc[:, :, 0:1].squeeze(2), op=ALU.mult)


def _col(ap3, g, j):
    """[128, G, J] tile -> [128, 1] scalar AP at (g, j)."""
    return ap3[:, g : g + 1, j : j + 1].squeeze(2)


def build_program(nvc=NVC):
    _register_scale2()
    nchunk = nvc // VCHUNK
    nc = bacc.Bacc("TRN2", target_bir_lowering=False, debug=False)

    verts_d = nc.dram_tensor("verts", [nvc, 3], F32, kind="ExternalInput")
    table_d = nc.dram_tensor("table", [CELLS * ESC], F16, kind="ExternalInput")
    msum_a_d = nc.dram_tensor("msum_a", [128, 9, C], MM_DT, kind="ExternalInput")
    msum_b_d = nc.dram_tensor("msum_b", [128, 9, C], MM_DT, kind="ExternalInput")
    msum_c_d = nc.dram_tensor("msum_c", [4, 9, C], MM_DT, kind="ExternalInput")
    wsh_a_d = nc.dram_tensor("wsh_a", [128, 3 * NN], MM_DT, kind="ExternalInput")
    wsh_b_d = nc.dram_tensor("wsh_b", [128, 3 * NN], MM_DT, kind="ExternalInput")
    wsh_c_d = nc.dram_tensor("wsh_c", [4, 3 * NN], MM_DT, kind="ExternalInput")
    rep16_d = nc.dram_tensor("rep16", [16, 128], F32, kind="ExternalInput")
    out_d = nc.dram_tensor("out", [nvc, C], F32, kind="ExternalOutput")

    tbl_ap = bass.AP(table_d, 0, [[ESC, CELLS], [1, ESC]])

    with tile.TileContext(nc) as tc:
        with (
            tc.tile_pool(name="const", bufs=1) as cst,
            tc.tile_pool(name="wts", bufs=1) as wp,
            tc.tile_pool(name="ix", bufs=2) as ixp,
            tc.tile_pool(name="gatc", bufs=2) as gcp,
            tc.tile_pool(name="gatn", bufs=3) as gnp,
            tc.tile_pool(name="blend", bufs=3) as bp,
            tc.tile_pool(name="feat", bufs=4) as fp,
            tc.tile_pool(name="chk", bufs=3) as kp,
            tc.tile_pool(name="misc", bufs=2) as mp,
            tc.tile_pool(name="dram", bufs=2, space="DRAM") as dp,
            tc.tile_pool(name="pso", bufs=1, space="PSUM") as pso,
            tc.tile_pool(name="pst", bufs=2, space="PSUM") as pst,
            tc.tile_pool(name="pss", bufs=1, space="PSUM") as pss,
            tc.tile_pool(name="psr", bufs=1, space="PSUM") as psr,
        ):
            ident = cst.tile([128, 128], F16)
            make_identity(nc, ident[:])
            msum_a = cst.tile([128, 9, C], MM_DT)
            msum_b = cst.tile([128, 9, C], MM_DT)
            msum_c = cst.tile([4, 9, C], MM_DT)
            wsh_a = cst.tile([128, 3 * NN], MM_DT)
            wsh_b = cst.tile([128, 3 * NN], MM_DT)
            wsh_c = cst.tile([4, 3 * NN], MM_DT)
            rep16 = cst.tile([16, 128], F32)
            nc.sync.dma_start(msum_a[:], msum_a_d[:])
            nc.sync.dma_start(msum_b[:], msum_b_d[:])
            nc.sync.dma_start(msum_c[:], msum_c_d[:])
            nc.sync.dma_start(wsh_a[:], wsh_a_d[:])
            nc.sync.dma_start(wsh_b[:], wsh_b_d[:])
            nc.sync.dma_start(wsh_c[:], wsh_c_d[:])
            nc.sync.dma_start(rep16[:], rep16_d[:])

            verts = cst.tile([128, nvc // 128, 3], F32)
            nc.sync.dma_start(
                verts[:], verts_d[:].rearrange("(vt p) c -> p vt c", p=128))

            def replicate_idx(scr_ap, n, tag):
                """n indices (n%512==0) from DRAM scratch (wrapped-16 blocks
                of 512) -> [128, n//16] idx tile replicated across all
                16-partition groups (each GPSIMD core pair reads its own)."""
                t16 = ixp.tile([16, n // 16], I16, tag=f"i16{tag}")
                nc.sync.dma_start(
                    t16[:], scr_ap.rearrange("(b m q) -> q (b m)", q=16, m=32))
                f16t = ixp.tile([16, n // 16], F32, tag=f"if{tag}")
                nc.vector.tensor_copy(f16t[:], t16[:])
                pr = psr.tile([128, n // 16], F32, space="PSUM", tag="rep",
                              name=f"pr{tag}")
                nc.tensor.matmul(pr[:], rep16[:], f16t[:], start=True, stop=True)
                it = kp.tile([128, n // 16], I16, tag=f"idx{tag}")
                nc.vector.tensor_copy(it[:], pr[:])
                return it

            # ---- whole-core center index math ----
            r16c = wp.tile([128, nvc // 128], I16)
            w8c = wp.tile([128, nvc // 128, 8], F32)
            _emit_index_math(nc, wp, verts[:], nvc // 128, r16c[:], w8c[:])
            scr_c = dp.tile([nvc], I16)
            nc.sync.dma_start(
                scr_c[:].rearrange("(vt p) -> p vt", p=128), r16c[:])
            # idx tiles for centers, one [128, 32] slice per chunk
            idx_c = []
            for vc in range(0, nvc // VCHUNK, 8):
                nblk = min(8, nvc // VCHUNK - vc)
                it = replicate_idx(
                    scr_c[vc * VCHUNK : (vc + nblk) * VCHUNK],
                    nblk * VCHUNK, f"c{vc}")
                idx_c += [it[:, k * 32 : (k + 1) * 32] for k in range(nblk)]

            def gather512(idx_ap, pool):
                gt = pool.tile([128, GPC, ESC], F16, tag="g")
                nc.gpsimd.dma_gather(
                    gt[:], tbl_ap, idx_ap, VCHUNK, VCHUNK, ESC)
                return gt

            def blend_group(gt, g, wap):
                """One 128-pt group: 8 corner slices -> blended [128, C] f16.
                wap: [128, 1, 8] per-corner weight AP for this group."""
                feat = fp.tile([128, C + 4], F16, tag="feat")
                terms = []
                for m in range(4):
                    tt = bp.tile([128, C], F16, tag=f"t{m % 2}")
                    nc.vector._custom_dve(
                        _SCALE2, out=tt[:],
                        in0=gt[:, g, (2 * m) * C : (2 * m + 1) * C],
                        in1=gt[:, g, (2 * m + 1) * C : (2 * m + 2) * C],
                        s0=_col(wap, 0, 2 * m), s1=_col(wap, 0, 2 * m + 1))
                    terms.append(tt)
                    if m == 1:
                        nc.vector.tensor_tensor(
                            terms[0][:], terms[0][:], terms[1][:], op=ALU.add)
                    if m == 3:
                        nc.vector.tensor_tensor(
                            terms[2][:], terms[2][:], terms[3][:], op=ALU.add)
                nc.vector.tensor_tensor(
                    feat[:, 0:C], terms[0][:], terms[2][:], op=ALU.add)
                return feat

            def finish_feat(feat, coords_ap, pool, tagsuf=""):
                """append [coords, 1] then transpose -> (fT0, fT1, fT2)."""
                nc.vector.tensor_copy(feat[:, C : C + 3], coords_ap)
                nc.vector.memset(feat[:, C + 3 : C + 4], 1.0)
                fts = []
                for ch, (lo, w) in enumerate(((0, 128), (128, 128), (C, 4))):
                    pt = pst.tile([128, 128], F16, space="PSUM", tag="pt",
                                  name=f"pt{ch}{tagsuf}")
                    nc.tensor.transpose(
                        pt[:w, :], feat[:, lo : lo + w], ident[:])
                    ft = pool.tile([w, 128], MM_DT, tag=f"ft{ch}{tagsuf}",
                                   name=f"ft{ch}{tagsuf}")
                    nc.scalar.copy(ft[:], pt[:w, :])
                    fts.append(ft)
                return fts

            state = {}  # per-chunk: center fts, ncoord, w8n, idx_n

            def centerA(vc):
                gts = gather512(idx_c[vc], gcp)
                ncoord = kp.tile([128, GPC, NN, 3], F32, tag="ncrd")
                fts_c = []
                for g in range(GPC):
                    vt = vc * GPC + g
                    feat = blend_group(gts, g, w8c[:, vt : vt + 1, :])
                    fts = finish_feat(feat, verts[:, vt, :], kp, f"c{g}")
                    fts_c.append(fts)
                    # shift matmul -> [128 pts, 24]
                    sps = pss.tile([128, 3 * NN], F32, space="PSUM", tag="sh")
                    for ch, rhs in enumerate((wsh_a, wsh_b, wsh_c)):
                        nc.tensor.matmul(
                            sps[:], fts[ch][:], rhs[:],
                            start=(ch == 0), stop=(ch == 2))
                    ssb = mp.tile([128, 3 * NN], F32, tag="ssb")
                    nc.scalar.copy(ssb[:], sps[:])
                    # neighbour coords: verts + shift  [128, NN, 3]
                    nc.vector.tensor_tensor(
                        ncoord[:, g, :, :],
                        ssb[:].rearrange("p (nn c) -> p nn c", c=3),
                        verts[:, vt : vt + 1, :].to_broadcast([128, NN, 3]),
                        op=ALU.add)
                # ---- neighbour index math (whole chunk) ----
                r16n = ixp.tile([128, GPC * NN], I16, tag="r16n")
                w8n = kp.tile([128, GPC * NN, 8], F32, tag="w8n")
                _emit_index_math(
                    nc, ixp,
                    ncoord[:].rearrange("p g nn c -> p (g nn) c"),
                    GPC * NN, r16n[:], w8n[:])
                scr_n = dp.tile([NN * VCHUNK], I16, tag="scrn")
                # block nn holds order (g, p); one 2D DMA per nn
                r16n_v = r16n[:].rearrange("p (g nn) -> p g nn", nn=NN)
                for nn_i in range(NN):
                    nc.sync.dma_start(
                        scr_n[nn_i * VCHUNK : (nn_i + 1) * VCHUNK].rearrange(
                            "(g p) -> p g", p=128),
                        r16n_v[:, :, nn_i])
                idx_n = replicate_idx(scr_n[:], NN * VCHUNK, "n")
                state[vc] = (fts_c, ncoord, w8n, idx_n)

            def neighborsB(vc):
                fts_c, ncoord, w8n, idx_n = state.pop(vc)
                out_ps = [
                    pso.tile([128, C], F32, space="PSUM", tag=f"o{g}",
                             name=f"ops{vc}_{g}")
                    for g in range(GPC)
                ]
                # center k=0 from stored transposed features
                for g in range(GPC):
                    for ch, rhs in enumerate((msum_a, msum_b, msum_c)):
                        nc.tensor.matmul(
                            out_ps[g][:], fts_c[g][ch][:], rhs[:, 0, :],
                            start=(ch == 0), stop=False)
                for nn_i in range(NN):
                    gtn = gather512(idx_n[:, nn_i * 32 : (nn_i + 1) * 32], gnp)
                    for g in range(GPC):
                        feat = blend_group(
                            gtn, g, w8n[:, g * NN + nn_i : g * NN + nn_i + 1, :])
                        fts = finish_feat(feat, ncoord[:, g, nn_i, :], fp)
                        for ch, rhs in enumerate((msum_a, msum_b, msum_c)):
                            nc.tensor.matmul(
                                out_ps[g][:], fts[ch][:],
                                rhs[:, nn_i + 1, :], start=False,
                                stop=(nn_i == NN - 1 and ch == 2))
                # ---- epilogue ----
                for g in range(GPC):
                    osb = mp.tile([128, C], F32, tag="osb")
                    nc.scalar.copy(osb[:], out_ps[g][:])
                    lo = (vc * GPC + g) * 128
                    nc.sync.dma_start(out_d[lo : lo + 128, :], osb[:])

            centerA(0)
            for vc in range(nchunk):
                if vc + 1 < nchunk:
                    centerA(vc + 1)
                neighborsB(vc)

    nc.compile()
    return nc


# --------------------------------------------------------------- host wrapper
_CACHED = {}


def _host_prep(x, W_shift, b_shift, W_diff, b_diff, W_center, b_center,
               W_sum, b_sum):
    # fp16 cell table per batch: cell (z,y,x) -> 8 corners x 256 ch contiguous
    xt = np.ascontiguousarray(
        np.transpose(x, (0, 2, 3, 4, 1))).astype(np.float16)   # [B,D,H,W,C]
    xp = np.pad(xt, ((0, 0), (0, 1), (0, 1), (0, 1), (0, 0)), mode="edge")
    cell = np.empty((B, GRID, GRID, GRID, 8, C), np.float16)
    for ci, (dz, dy, dx) in enumerate(
        [(z, y, xx) for z in (0, 1) for y in (0, 1) for xx in (0, 1)]):
        cell[:, :, :, :, ci, :] = xp[:, dz : dz + GRID, dy : dy + GRID,
                                     dx : dx + GRID, :]
    table = cell.reshape(B, CELLS * ESC)

    M = np.einsum("ock,cd->okd", W_sum.astype(np.float64),
                  W_diff.astype(np.float64))                  # [256, 9, 259]
    M = np.transpose(M, (1, 0, 2))                            # [9, 256, 259]
    M = M.copy()
    M[0] += W_center.astype(np.float64)
    bias = (W_sum.astype(np.float64).sum(-1) @ b_diff.astype(np.float64)
            + b_sum + b_center)                               # [256]
    msum = np.zeros((9, C + 4, C), np.float16)
    for k in range(9):
        msum[k, : C + 3, :] = M[k].T.astype(np.float16)
    msum[0, C + 3, :] = bias.astype(np.float16)
    msum_a = np.ascontiguousarray(np.transpose(msum[:, 0:128, :], (1, 0, 2)))
    msum_b = np.ascontiguousarray(np.transpose(msum[:, 128:256, :], (1, 0, 2)))
    msum_c = np.ascontiguousarray(np.transpose(msum[:, 256:260, :], (1, 0, 2)))

    wsh = np.zeros((C + 4, 3 * NN), np.float16)
    wsh[0:C, :] = W_shift.T.astype(np.float16)
    wsh[C + 3, :] = b_shift.astype(np.float16)
    return table, msum_a, msum_b, msum_c, wsh


def kernel(x, vertices, W_shift, b_shift, W_diff, b_diff, W_center, b_center,
           W_sum, b_sum):
    if "nc" not in _CACHED:
        _CACHED["nc"] = build_program()
    nc = _CACHED["nc"]

    table, msum_a, msum_b, msum_c, wsh = _host_prep(
        x, W_shift, b_shift, W_diff, b_diff, W_center, b_center, W_sum, b_sum)
    wsh_a, wsh_b, wsh_c = wsh[0:128], wsh[128:256], wsh[256:260]

    in_maps = []
    for core in range(8):
        b, h = divmod(core, 2)
        in_maps.append({
            "verts": np.ascontiguousarray(
                vertices[b, h * NVC : (h + 1) * NVC]).astype(np.float32),
            "table": table[b],
            "msum_a": msum_a, "msum_b": msum_b, "msum_c": msum_c,
            "wsh_a": np.ascontiguousarray(wsh_a),
            "wsh_b": np.ascontiguousarray(wsh_b),
            "wsh_c": np.ascontiguousarray(wsh_c),
            "rep16": np.tile(np.eye(16, dtype=np.float32), 8),
        })

    res = run_bass_kernel_spmd(nc, in_maps, core_ids=list(range(8)))
    out = np.empty((B, N, C), np.float32)
    for core in range(8):
        b, h = divmod(core, 2)
        out[b, h * NVC : (h + 1) * NVC] = res.results[core]["out"]
    return out
